# revision 7
# baseline (speedup 1.0000x reference)
"""Bass/Trainium2 kernel for nn_CasualSelfAttention (B=4, T=2048, D=1024, H=16, dk=64).

Sharding: batch (4) x t-half (2) = 8 cores; core (b, h) computes output rows
t in [h*1024, (h+1)*1024) of batch b for ALL 16 heads. In this module's
(swapped) attention, out rows index KEYS and softmax runs over QUERIES, so
each core needs its own half of the key projections but the FULL query and
value projections. Each core therefore receives only its t-half of X^T
(every activation byte ships to exactly one core), projects q/v for its
half, and a pair AllGather (cores 2b, 2b+1) assembles the full q and V.
Weights are baked into the program as fp16 constants (shipped as XLA
literals, not per-core inputs); outputs are disjoint fp16 t-halves, so the
host does no cross-core reduction.

All wire + matmul dtypes are fp16 (f32 PSUM accumulation); measured
absmax-relative error vs the f64 reference is ~5e-3 (budget 2e-2). Softmax
sums use a V-augmented ones column (M=65 PV matmuls); normalization is a DVE
reciprocal plus a K=1 f32r PE broadcast matmul, as in the earlier head-split
kernel this derives from.
"""
import sys
import os

sys.path.insert(0, '/opt/trn_rl_repo')

import hashlib
import numpy as np
import orjson

import concourse.bass as bass
import concourse.tile as tile
import concourse.mybir as mybir
from concourse.bass_utils import run_bass_kernel_spmd

# ---------------------------------------------------------------- waitsplit
# The walrus build in this container accepts at most ONE semaphore wait per
# engine instruction.  Tile emits multi-wait sync_info; split the extras into
# single-wait NoOps on the same engine stream (in-order => semantically equal).
_ws_counter = [0]


_SELF_WAIT_ENGINES = ("Activation", "DVE")


def _split_instruction_waits(inst, out_list):
    si = inst.get("sync_info")
    if not si or not si.get("on_wait"):
        out_list.append(inst)
        return
    waits = si["on_wait"]
    # ACT/DVE execute strictly in order, so a compute instruction's wait on
    # its OWN engine's semaphore (slot-reuse WAW vs an older instruction on
    # the same engine) is always already satisfied — drop it instead of
    # spending a NoOp dispatch on the bottleneck ACT stream.
    eng = inst.get("engine")
    if (eng in _SELF_WAIT_ENGINES
            and inst.get("opcode") not in ("Drain", "EventSemaphore", "NoOp")):
        kept = [w for w in waits
                if w.get("ant_name", "").rsplit("_", 1)[0] != eng]
        if kept != waits:
            si = dict(si)
            si["on_wait"] = kept
            inst = dict(inst)
            inst["sync_info"] = si
            waits = kept
    if len(waits) <= 1:
        out_list.append(inst)
        return
    for w in waits[:-1]:
        _ws_counter[0] += 1
        out_list.append({
            "debug": inst.get("debug", 0),
            "engine": inst.get("engine"),
            "ins": [],
            "name": f"I-wsplit-{_ws_counter[0]}",
            "opcode": "NoOp",
            "outs": [],
            "sync_info": {"on_update": [], "on_wait": [w]},
        })
    si = dict(si)
    si["on_wait"] = [waits[-1]]
    inst = dict(inst)
    inst["sync_info"] = si
    out_list.append(inst)


def fix_multiwait_json(bir_bytes):
    d = orjson.loads(bir_bytes)
    for fn in d["functions"]:
        for bb in fn["blocks"]:
            new = []
            for inst in bb["instructions"]:
                _split_instruction_waits(inst, new)
            bb["instructions"] = new
    return orjson.dumps(d)


class WaitSplitBass(bass.Bass):
    def to_json_bytes(self):
        return fix_multiwait_json(super().to_json_bytes())


# ---------------------------------------------------------------- kernel build
P = 128
B, T, D = 4, 2048, 1024
NH = 16               # heads per core (all of them)
NP = NH // 2          # head pairs per core = 8
DK = 64
DC = D // P           # 8 d_model chunks
SC = T // P           # 16 s-chunks (full T: softmax axis)
TH = T // 2           # local t rows per core
NTB = TH // 512       # 2 t-blocks of local rows
f16 = mybir.dt.float16
f32 = mybir.dt.float32
f32r = mybir.dt.float32r
AF = mybir.ActivationFunctionType
MULT = mybir.AluOpType.mult
RG = [[0, 1], [2, 3], [4, 5], [6, 7]]
SALT = "w2a"          # bump on any rebuild after a mesh desync

_nc_cache = {}


def build_nc(WQ_w, WQ_b, WK_w, WK_b, WV_w, WV_b, WO_w):
    wqT = np.ascontiguousarray(np.asarray(WQ_w, np.float32).T).astype(np.float16)
    wkT = np.ascontiguousarray(np.asarray(WK_w, np.float32).T).astype(np.float16)
    wvT = np.ascontiguousarray(np.asarray(WV_w, np.float32).T).astype(np.float16)
    woT = np.ascontiguousarray(np.asarray(WO_w, np.float32).T).astype(np.float16)
    bq = np.ascontiguousarray(
        np.asarray(WQ_b, np.float32).reshape(NP, P).T)
    bk = np.ascontiguousarray(
        np.asarray(WK_b, np.float32).reshape(NP, P).T)
    key = hashlib.sha256(
        b"".join(a.tobytes() for a in (wqT, wkT, wvT, woT, bq, bk))).hexdigest()
    if key in _nc_cache:
        return _nc_cache[key]

    nc = WaitSplitBass()
    xin = nc.dram_tensor(f"xin_{SALT}", [3, D, TH], f16, kind="ExternalInput")
    out = nc.dram_tensor(f"out_{SALT}", [TH, D], f16, kind="ExternalOutput")

    with tile.TileContext(nc) as tc:
        wq_c = nc.inline_tensor(wqT, name=f"wq_{SALT}")
        wk_c = nc.inline_tensor(wkT, name=f"wk_{SALT}")
        wv_c = nc.inline_tensor(wvT, name=f"wv_{SALT}")
        wo_c = nc.inline_tensor(woT, name=f"wo_{SALT}")
        bq_c = nc.inline_tensor(bq, name=f"bq_{SALT}")
        bk_c = nc.inline_tensor(bk, name=f"bk_{SALT}")
        with tc.tile_pool(name="persist", bufs=1) as persist, \
             tc.tile_pool(name="psS", bufs=2, space="PSUM") as psS, \
             tc.tile_pool(name="psProj", bufs=2, space="PSUM") as psProj, \
             tc.tile_pool(name="psPV", bufs=1, space="PSUM") as psPV:

            # ---- persistent tiles ----
            qT2 = [persist.tile([P, T], f16, tag=f"qT2_{p}", name=f"qT2_{p}")
                   for p in range(NP)]
            kT2 = [persist.tile([P, TH], f16, tag=f"kT2_{p}", name=f"kT2_{p}")
                   for p in range(NP)]
            V_aug = persist.tile([P, SC, NH, 65], f16, name="V_aug")
            nc.vector.memset(V_aug[:, :, :, 64], 1.0)
            bq_s = persist.tile([P, NP], f32, name="bq_s")
            nc.sync.dma_start(bq_s[:], bq_c[:])
            bk_s = persist.tile([P, NP], f32, name="bk_s")
            nc.sync.dma_start(bk_s[:], bk_c[:])
            ones64 = persist.tile([1, 64], f32r, name="ones64")
            nc.vector.memset(ones64[:].bitcast(f32), 1.0)

            # ---- phase A: projections (half-t q/v/k) + pair AllGather ----
            # 8 concurrent one-bank psum accumulators: psS's two [P,1024]
            # tiles sliced in half (4) + psProj (2) + psPV's pv slots (2).
            def alloc8(stem):
                ps = []
                for j in range(2):
                    big = psS.tile([P, 1024], f32, tag="scores",
                                   name=f"{stem}_s{j}")
                    ps.append(big[:, 0:512])
                    ps.append(big[:, 512:1024])
                ps.append(psProj.tile([P, 512], f32, tag="proj",
                                      name=f"{stem}_p0"))
                ps.append(psProj.tile([P, 512], f32, tag="proj",
                                      name=f"{stem}_p1"))
                ps.append(psPV.tile([P, 512], f32, tag="pv0",
                                    name=f"{stem}_v0"))
                ps.append(psPV.tile([P, 512], f32, tag="pv1",
                                    name=f"{stem}_v1"))
                return ps

            with tc.tile_pool(name="dramA", bufs=1, space="DRAM") as dramA, \
                 tc.tile_pool(name="wpool", bufs=1) as wpool, \
                 tc.tile_pool(name="xpool", bufs=8) as xpool, \
                 tc.tile_pool(name="stpool", bufs=8) as stpool:
                gq_in = dramA.tile([D, TH], f16, name=f"gqi_{SALT}")
                gq_out = dramA.tile([2, D, TH], f16, name=f"gqo_{SALT}")
                gv_in = dramA.tile([TH, D], f16, name=f"gvi_{SALT}")
                gv_out = dramA.tile([2, TH, D], f16, name=f"gvo_{SALT}")

                wq = wpool.tile([P, DC, 1024], f16, tag="wq", name="wq")
                wk = wpool.tile([P, DC, 1024], f16, tag="wk", name="wk")
                wv = wpool.tile([P, DC, 1024], f16, tag="wv", name="wv")
                for c in range(DC):
                    nc.sync.dma_start(wq[:, c], wq_c[c * P:(c + 1) * P, :])
                    nc.sync.dma_start(wk[:, c], wk_c[c * P:(c + 1) * P, :])
                    nc.sync.dma_start(wv[:, c], wv_c[c * P:(c + 1) * P, :])

                # Q projection: psum [hc, t]; out blocks (hcb 0..7) x (xb 0..1)
                for xb in range(2):
                    ts_ = slice(xb * 512, (xb + 1) * 512)
                    ps8 = alloc8(f"psq{xb}")
                    for c in range(DC):
                        x_c = xpool.tile([P, 512], f16, tag="xq",
                                         name=f"xq_{xb}_{c}")
                        nc.sync.dma_start(x_c[:], xin[0, c * P:(c + 1) * P, ts_])
                        for hcb in range(NP):
                            nc.tensor.matmul(
                                ps8[hcb][:], wq[:, c, hcb * P:(hcb + 1) * P],
                                x_c[:], start=(c == 0), stop=(c == DC - 1))
                    for hcb in range(NP):
                        st = stpool.tile([P, 512], f16, tag="stq",
                                         name=f"stq_{xb}_{hcb}")
                        nc.vector.tensor_scalar_add(
                            st[:], ps8[hcb][:], bq_s[:, hcb:hcb + 1])
                        nc.sync.dma_start(
                            gq_in[hcb * P:(hcb + 1) * P, ts_], st[:])
                nc.gpsimd.collective_compute(
                    "AllGather", mybir.AluOpType.bypass, replica_groups=RG,
                    ins=[gq_in[:]], outs=[gq_out[:]])

                # V projection: psum [s, hc]; out blocks (st 0..7) x (hch 0..1)
                for sb in range(2):
                    ps8 = alloc8(f"psv{sb}")
                    for c in range(DC):
                        xv_c = xpool.tile([P, 512], f16, tag="xv",
                                          name=f"xv_{sb}_{c}")
                        nc.sync.dma_start(
                            xv_c[:], xin[2, c * P:(c + 1) * P,
                                         sb * 512:(sb + 1) * 512])
                        for stl in range(4):
                            for hch in range(2):
                                nc.tensor.matmul(
                                    ps8[stl * 2 + hch][:],
                                    xv_c[:, stl * P:(stl + 1) * P],
                                    wv[:, c, hch * 512:(hch + 1) * 512],
                                    start=(c == 0), stop=(c == DC - 1))
                    for stl in range(4):
                        for hch in range(2):
                            st = stpool.tile([P, 512], f16, tag="stv",
                                             name=f"stv_{sb}_{stl}_{hch}")
                            nc.vector.tensor_copy(st[:], ps8[stl * 2 + hch][:])
                            nc.sync.dma_start(
                                gv_in[(sb * 4 + stl) * P:(sb * 4 + stl + 1) * P,
                                      hch * 512:(hch + 1) * 512], st[:])
                nc.gpsimd.collective_compute(
                    "AllGather", mybir.AluOpType.bypass, replica_groups=RG,
                    ins=[gv_in[:]], outs=[gv_out[:]])

                # K projection: psum [hc, t] -> kT2 directly (local half only)
                for xb in range(2):
                    ts_ = slice(xb * 512, (xb + 1) * 512)
                    ps8 = alloc8(f"psk{xb}")
                    for c in range(DC):
                        xk_c = xpool.tile([P, 512], f16, tag="xk",
                                          name=f"xk_{xb}_{c}")
                        nc.sync.dma_start(xk_c[:], xin[1, c * P:(c + 1) * P, ts_])
                        for hcb in range(NP):
                            nc.tensor.matmul(
                                ps8[hcb][:], wk[:, c, hcb * P:(hcb + 1) * P],
                                xk_c[:], start=(c == 0), stop=(c == DC - 1))
                    for hcb in range(NP):
                        nc.vector.tensor_scalar_add(
                            kT2[hcb][:, ts_], ps8[hcb][:], bk_s[:, hcb:hcb + 1])

                # gather results into SBUF
                for g in range(2):
                    for p in range(NP):
                        nc.sync.dma_start(
                            qT2[p][:, g * TH:(g + 1) * TH],
                            gq_out[g, p * P:(p + 1) * P, :])
                    for scl in range(8):
                        sc = g * 8 + scl
                        nc.sync.dma_start(
                            V_aug[:, sc, :, 0:64],
                            gv_out[g, scl * P:(scl + 1) * P, :]
                            .rearrange("p (h d) -> p h d", d=64))

            # ---- phases B+C per t-block ----
            with tc.tile_pool(name="ppool", bufs=5) as ppool, \
                 tc.tile_pool(name="rbpool", bufs=2) as rbpool, \
                 tc.tile_pool(name="ctxpool", bufs=2) as ctxpool, \
                 tc.tile_pool(name="wopool", bufs=1) as wopool, \
                 tc.tile_pool(name="opool", bufs=3) as opool:
                    wo = wopool.tile([P, NP, D], f16, name="wo")
                    for p in range(NP):
                        nc.sync.dma_start(wo[:, p], wo_c[p * P:(p + 1) * P, :])

                    def flush_evac(pend):
                        # normalize pair into its ctx tile:
                        # ctx[h] = pv[h][0:64] * bcast(1 / pv[h][64])
                        tb, p, pv0, pv1, ctx_p = pend
                        for h, pv in ((0, pv0), (1, pv1)):
                            r_t = rbpool.tile([1, 512], f32r, tag="r",
                                              name=f"r_{tb}_{p}_{h}")
                            with nc.allow_low_precision(reason="softmax recip"):
                                nc.vector.reciprocal(r_t[:], pv[64:65, :])
                            ps_rb = psProj.tile([64, 512], f32, tag="proj",
                                                name=f"ps_rb_{tb}_{p}_{h}")
                            nc.tensor.matmul(ps_rb[:], ones64[:], r_t[:],
                                             start=True, stop=True)
                            rb_s = rbpool.tile([64, 512], f32, tag="rb",
                                               name=f"rb_{tb}_{p}_{h}")
                            nc.vector.tensor_copy(rb_s[:], ps_rb[:])
                            nc.vector.tensor_tensor(
                                ctx_p[h * 64:(h + 1) * 64, :],
                                pv[0:64, :], rb_s[:], MULT)

                    def emit_wo_chunk(wtb, wctx, ti, ob):
                        # one [128t, 512o] WO output tile of t-block wtb
                        ps_o = psProj.tile([P, 512], f32, tag="proj",
                                           name=f"ps_o_{wtb}_{ti}_{ob}")
                        for p in range(NP):
                            nc.tensor.matmul(
                                ps_o[:], wctx[p][:, ti * P:(ti + 1) * P],
                                wo[:, p, ob * 512:(ob + 1) * 512],
                                start=(p == 0), stop=(p == NP - 1))
                        o_t = opool.tile([P, 512], f16, tag="o",
                                         name=f"o_{wtb}_{ti}_{ob}")
                        nc.vector.tensor_copy(o_t[:], ps_o[:])
                        nc.sync.dma_start(
                            out[wtb * 512 + ti * P: wtb * 512 + (ti + 1) * P,
                                ob * 512:(ob + 1) * 512], o_t[:])

                    pending = None        # (tb, p, pv0, pv1, ctx) to normalize
                    pending_wo = None     # (tb, ctx_tb) whose WO is deferred
                    for tb in range(NTB):
                        ts_ = slice(tb * 512, (tb + 1) * 512)
                        ctx_tb = []
                        for p in range(NP):
                            # -- B: attention for (pair p, t-block tb) --
                            pv0 = psPV.tile([65, 512], f32, tag="pv0",
                                            name=f"pv0_{tb}_{p}")
                            pv1 = psPV.tile([65, 512], f32, tag="pv1",
                                            name=f"pv1_{tb}_{p}")
                            # Defer the previous pair's PV-psum evacuation (and
                            # the previous t-block's WO chunks) past this
                            # pair's first score/exp groups so ACT stays fed
                            # while PE runs the evac/WO work in its slack.
                            stash = []
                            for sc in range(SC):
                                ss = slice(sc * P, (sc + 1) * P)
                                ps_s = psS.tile([P, 1024], f32, tag="scores",
                                                name=f"ps_s_{tb}_{p}_{sc}")
                                nc.tensor.matmul(
                                    ps_s[:, 0:512], qT2[p][0:64, ss],
                                    kT2[p][0:64, ts_], start=True, stop=True,
                                    tile_position=(0, 0))
                                nc.tensor.matmul(
                                    ps_s[:, 512:1024], qT2[p][64:128, ss],
                                    kT2[p][64:128, ts_], start=True, stop=True,
                                    tile_position=(64, 0))
                                p_t = ppool.tile([P, 1024], f16, tag="p",
                                                 name=f"p_{tb}_{p}_{sc}")
                                nc.scalar.activation(p_t[:], ps_s[:], AF.Exp,
                                                     scale=0.125)
                                if pending is not None and sc < 2:
                                    stash.append((sc, p_t))
                                    continue
                                if pending is not None and sc == 2:
                                    flush_evac(pending)
                                    pending = None
                                for s0, pt0 in stash:
                                    nc.tensor.matmul(
                                        pv0[:], V_aug[:, s0, 2 * p, :],
                                        pt0[:, 0:512],
                                        start=(s0 == 0), stop=False)
                                    nc.tensor.matmul(
                                        pv1[:], V_aug[:, s0, 2 * p + 1, :],
                                        pt0[:, 512:1024],
                                        start=(s0 == 0), stop=False)
                                stash = []
                                nc.tensor.matmul(
                                    pv0[:], V_aug[:, sc, 2 * p, :],
                                    p_t[:, 0:512],
                                    start=(sc == 0), stop=(sc == SC - 1))
                                nc.tensor.matmul(
                                    pv1[:], V_aug[:, sc, 2 * p + 1, :],
                                    p_t[:, 512:1024],
                                    start=(sc == 0), stop=(sc == SC - 1))
                                # sprinkle the previous t-block's 8 WO
                                # chunks across pairs 0-1, every other sc,
                                # to stay under the ACT rate per slot
                                if (pending_wo is not None and p <= 1
                                        and 2 <= sc <= 9 and (sc % 2) == 0):
                                    widx = p * 4 + (sc - 2) // 2
                                    emit_wo_chunk(pending_wo[0], pending_wo[1],
                                                  widx // 2, widx % 2)
                                    if widx == 7:
                                        pending_wo = None
                            ctx_p = ctxpool.tile([P, 512], f16, tag=f"ctx{p}",
                                                 name=f"ctx_{tb}_{p}")
                            if pending is not None:
                                flush_evac(pending)
                            pending = (tb, p, pv0, pv1, ctx_p)
                            ctx_tb.append(ctx_p)
                        pending_wo = (tb, ctx_tb)

                    # tail: last pair's evac + last t-block's WO
                    if pending is not None:
                        flush_evac(pending)
                        pending = None
                    if pending_wo is not None:
                        for ti in range(4):
                            for ob in range(2):
                                emit_wo_chunk(pending_wo[0], pending_wo[1],
                                              ti, ob)
                        pending_wo = None
    _nc_cache[key] = nc
    return nc


# ---------------------------------------------------------------- execution
# One jitted program per built nc, shared by kernel() and any external timing
# harness. Collective-bearing NEFFs desync the axon mesh if a second
# executable instance is loaded in the same process, so everything in a
# process must run through this single program.
_exec_cache = {}


def get_executor(nc):
    if id(nc) in _exec_cache:
        return _exec_cache[id(nc)]
    import jax
    from jax.sharding import Mesh, PartitionSpec
    from jax.experimental.shard_map import shard_map
    from concourse import bass2jax

    bass2jax.install_neuronx_cc_hook()
    partition_name = (nc.partition_id_tensor.name if nc.partition_id_tensor
                      else None)
    in_names, out_names, out_avals, zero_outs = [], [], [], []
    for alloc in nc.m.functions[0].allocations:
        if not isinstance(alloc, mybir.MemoryLocationSet):
            continue
        name = alloc.memorylocations[0].name
        if alloc.kind == "ExternalInput":
            if name != partition_name:
                in_names.append(name)
        elif alloc.kind == "ExternalOutput":
            out_names.append(name)
            shape = tuple(alloc.tensor_shape)
            dtype = mybir.dt.np(alloc.dtype)
            out_avals.append(jax.core.ShapedArray(shape, dtype))
            zero_outs.append(np.zeros(shape, dtype))
    n_params = len(in_names)
    n_outs = len(out_names)
    all_names = list(in_names) + list(out_names)
    if partition_name is not None:
        all_names.append(partition_name)

    def _body(*args):
        operands = list(args)
        if partition_name is not None:
            operands.append(bass2jax.partition_id_tensor())
        outs = bass2jax._bass_exec_p.bind(
            *operands, out_avals=tuple(out_avals), in_names=tuple(all_names),
            out_names=tuple(out_names), lowering_input_output_aliases=(),
            sim_require_finite=True, sim_require_nnan=True, nc=nc)
        return tuple(outs)

    n_cores = 8
    devices = jax.devices()[:n_cores]
    mesh = Mesh(np.asarray(devices), ("core",))
    in_specs = (PartitionSpec("core"),) * (n_params + n_outs)
    out_specs = (PartitionSpec("core"),) * n_outs
    donate = tuple(range(n_params, n_params + n_outs))
    sharded = jax.jit(shard_map(_body, mesh=mesh, in_specs=in_specs,
                                out_specs=out_specs, check_rep=False),
                      donate_argnums=donate, keep_unused=True)
    ex = {
        "sharded": sharded, "in_names": in_names, "out_names": out_names,
        "out_avals": out_avals, "zero_outs": zero_outs, "n_cores": n_cores,
        "jax": jax,
    }
    _exec_cache[id(nc)] = ex
    return ex


def run_cores(nc, in_maps):
    """Execute on 8 cores; returns list of per-core {out_name: np.ndarray}."""
    ex = get_executor(nc)
    jax = ex["jax"]
    n_cores = ex["n_cores"]
    concat_in = [np.concatenate([np.asarray(in_maps[c][nm])
                                 for c in range(n_cores)], axis=0)
                 for nm in ex["in_names"]]
    concat_zero = [np.zeros((n_cores * z.shape[0], *z.shape[1:]), z.dtype)
                   for z in ex["zero_outs"]]
    out_arrs = ex["sharded"](*concat_in, *concat_zero)
    return [
        {name: np.asarray(out_arrs[i]).reshape(n_cores,
                                               *ex["out_avals"][i].shape)[c]
         for i, name in enumerate(ex["out_names"])}
        for c in range(n_cores)
    ]


# ---------------------------------------------------------------- host side
def make_in_maps(keys, queries, values):
    keys = np.asarray(keys, dtype=np.float32)
    queries = np.asarray(queries, dtype=np.float32)
    values = np.asarray(values, dtype=np.float32)
    in_maps = []
    for c in range(8):
        b, h = c // 2, c % 2
        sl = slice(TH * h, TH * (h + 1))
        xin = np.empty((3, D, TH), np.float16)
        xin[0] = queries[b].T[:, sl]
        xin[1] = keys[b].T[:, sl]
        xin[2] = values[b].T[:, sl]
        in_maps.append({f"xin_{SALT}": xin})
    return in_maps


def kernel(keys, queries, values, pad_mask, WK_w, WK_b, WQ_w, WQ_b, WV_w, WV_b,
           WO_w, WO_b):
    nc = build_nc(WQ_w, WQ_b, WK_w, WK_b, WV_w, WV_b, WO_w)
    in_maps = make_in_maps(keys, queries, values)
    results = run_cores(nc, in_maps)
    # free-dim biases folded on host: WO_b directly; WV_b exactly via
    # WV_b @ WO_w^T (attention rows sum to 1).
    bias = (np.asarray(WO_b, np.float64)
            + np.asarray(WV_b, np.float64) @ np.asarray(WO_w, np.float64).T
            ).astype(np.float32)
    out = np.empty((B, T, D), np.float32)
    for c in range(8):
        b, h = c // 2, c % 2
        out[b, TH * h:TH * (h + 1)] = (
            results[c][f"out_{SALT}"].astype(np.float32) + bias)
    return out
